# revision 1
# baseline (speedup 1.0000x reference)
"""Causal MHA (B=4, T=2048, D=1024, H=16, Dh=64) on 8 TRN2 NeuronCores.

Sharding: tensor-parallel over heads (2 groups of 8 heads; W_q/W_k/W_v split
column-wise, W_o row-wise) x data-parallel over batch (4 batches). Core
c = (b, g) computes a partial output x[b] attention with head-group g; the
host sums the two head-group partials per batch.

v3 (bf16, software-pipelined): the host pre-transposes x and converts all
operands to bf16 (no on-device x transposes; every matmul at 1 cycle/row with
FWL). Emission is interleaved so attention (ACT-heavy) starts as soon as
head 0's projections land, and the remaining projection GEMMs + the output
projection act as PE filler during ACT-bound stretches:

  B: Q^T/K^T [I,T] via W-stationary GEMMs; V natural per-head + ones column.
  C: per q-strip of 1024 and head: S^T k-strips (bank-split matmuls, one exp
     per strip on ACT, scale folded), diagonal masked by lower-tri multiply
     on DVE. PV accumulates ctx in NATURAL [q, 65] layout (pt stationary,
     V_aug moving, 65-col matmuls, full 128 contraction) -- half the PE
     cycles of the transposed formulation. Softmax normalization is a single
     per-partition tensor_scalar divide on DVE.
  D: ctx transposed 128x128 on PE, out = ctxT.T Wo, PSUM->SBUF->DRAM fp32.
"""

import numpy as np
import ml_dtypes

import concourse.bass as bass
import concourse.mybir as mybir
import concourse.tile as tile
from concourse import bacc
from concourse.bass_utils import run_bass_kernel_spmd
from concourse.masks import make_identity

B, T, D = 4, 2048, 1024
H_TOT, DH = 16, 64
N_CORES = 8
HPC = 8                  # heads per core
I = HPC * DH             # 512: inner width per core
F32 = mybir.dt.float32
BF16 = mybir.dt.bfloat16
NP_BF16 = ml_dtypes.bfloat16
SCALE = float(DH) ** -0.5
QS_W = 1024              # q-strip width
N_QS = T // QS_W

_NC_CACHE = []


def _emit(nc, tc, ctx):
    xt_d = nc.dram_tensor("xt", [D, T], BF16, kind="ExternalInput")
    wq_d = nc.dram_tensor("wq", [D, I], BF16, kind="ExternalInput")
    wk_d = nc.dram_tensor("wk", [D, I], BF16, kind="ExternalInput")
    wv_d = nc.dram_tensor("wv", [D, I], BF16, kind="ExternalInput")
    wo_d = nc.dram_tensor("wo", [I, D], BF16, kind="ExternalInput")
    o_d = nc.dram_tensor("o", [T, D], F32, kind="ExternalOutput")

    o_view = o_d.ap().rearrange("(n p) d -> n p d", p=128)   # [16,128,1024]

    # ---- constants ----
    persist = ctx.enter_context(tc.tile_pool(name="persist", bufs=1))
    ident32 = persist.tile([128, 128], F32, tag="ident32")
    make_identity(nc, ident32[:])
    ident = persist.tile([128, 128], BF16, tag="ident")
    nc.vector.tensor_copy(ident[:], ident32[:])
    # keep mask for S^T diag tiles: ltri[i,j] = 1.0 if j >= i else 0.0
    ltri32 = persist.tile([128, 128], F32, tag="ltri32")
    nc.gpsimd.memset(ltri32[:], 1.0)
    nc.gpsimd.affine_select(
        out=ltri32[:], in_=ltri32[:], compare_op=mybir.AluOpType.is_ge,
        fill=0.0, base=0, pattern=[[1, 128]], channel_multiplier=-1,
    )
    ltri = persist.tile([128, 128], BF16, tag="ltri")
    nc.vector.tensor_copy(ltri[:], ltri32[:])

    # ---- input DMAs, spread over 4 DGE queues ----
    wpool = ctx.enter_context(tc.tile_pool(name="weights", bufs=1))
    wq_t = wpool.tile([128, 8, I], BF16, tag="wq")
    wk_t = wpool.tile([128, 8, I], BF16, tag="wk")
    wv_t = wpool.tile([128, 8, I], BF16, tag="wv")
    wo_t = wpool.tile([128, 4, D], BF16, tag="wo")
    xTpool = ctx.enter_context(tc.tile_pool(name="xT", bufs=1))
    xT = xTpool.tile([128, 8, T], BF16, tag="xT")
    xr = xt_d.ap().rearrange("(c p) t -> p c t", p=128)  # [128, 8, 2048]
    wq_r = wq_d.ap().rearrange("(c p) i -> p c i", p=128)
    wk_r = wk_d.ap().rearrange("(c p) i -> p c i", p=128)
    # ti=0 chunks first so head 0's projections can start immediately
    nc.scalar.dma_start(wq_t[:, :, 0:128], wq_r[:, :, 0:128])
    nc.scalar.dma_start(wk_t[:, :, 0:128], wk_r[:, :, 0:128])
    nc.gpsimd.dma_start(wv_t[:], wv_d.ap().rearrange("(c p) i -> p c i", p=128))
    for dcp in range(4):
        eng = nc.sync if dcp < 2 else nc.gpsimd
        eng.dma_start(xT[:, 2 * dcp:2 * dcp + 2, 0:1024],
                      xr[:, 2 * dcp:2 * dcp + 2, 0:1024])
    nc.scalar.dma_start(wq_t[:, :, 128:512], wq_r[:, :, 128:512])
    nc.scalar.dma_start(wk_t[:, :, 128:512], wk_r[:, :, 128:512])
    for dcp in range(4):
        nc.sync.dma_start(xT[:, 2 * dcp:2 * dcp + 2, 1024:2048],
                          xr[:, 2 * dcp:2 * dcp + 2, 1024:2048])
    nc.sync.dma_start(wo_t[:], wo_d.ap().rearrange("(c p) d -> p c d", p=128))

    qkpool = ctx.enter_context(tc.tile_pool(name="qk", bufs=1))
    qT = [qkpool.tile([128, T], BF16, tag=f"qT{i}", name=f"qT{i}") for i in range(4)]
    kT = [qkpool.tile([128, T], BF16, tag=f"kT{i}", name=f"kT{i}") for i in range(4)]
    v3pool = ctx.enter_context(tc.tile_pool(name="v3", bufs=1))
    v3 = [v3pool.tile([128, HPC, DH + 1], BF16, tag=f"v{t}", name=f"v{t}")
          for t in range(16)]
    for gt in range(16):
        nc.gpsimd.memset(v3[gt][:, :, DH:DH + 1], 1.0)

    # psum pools (shared across stages; 4+2+2 = 8 banks)
    psum_big = ctx.enter_context(tc.tile_pool(name="psum_big", bufs=2, space="PSUM"))
    psum_med = ctx.enter_context(tc.tile_pool(name="psum_med", bufs=2, space="PSUM"))
    psum_cps = ctx.enter_context(tc.tile_pool(name="psum_cps", bufs=2, space="PSUM"))

    ptpool = ctx.enter_context(tc.tile_pool(name="pt", bufs=24))
    recpool = ctx.enter_context(tc.tile_pool(name="rec", bufs=6))
    ctxcpool = ctx.enter_context(tc.tile_pool(name="ctxc", bufs=2))
    ctxTpool = ctx.enter_context(tc.tile_pool(name="ctxT", bufs=1))
    osbpool = ctx.enter_context(tc.tile_pool(name="osb", bufs=2))
    ctxT = ctxTpool.tile([128, 4, T], BF16, tag="ctxT")
    ctxc = [ctxcpool.tile([128, 8, I], BF16, tag="ctxc", name=f"ctxc{qs}")
            for qs in range(N_QS)]

    # ---- emission helpers ----
    def emit_qk(ti, th):
        t0 = th * 1024
        for w_t, dstT in ((wq_t, qT), (wk_t, kT)):
            for tb in range(2):  # one psum bank per matmul out
                ps = psum_med.tile([128, 512], F32, tag="med")
                for dc in range(8):
                    nc.tensor.matmul(
                        ps[:],
                        w_t[:, dc, ti * 128:(ti + 1) * 128],
                        xT[:, dc, t0 + tb * 512:t0 + (tb + 1) * 512],
                        start=(dc == 0), stop=(dc == 7),
                    )
                nc.vector.tensor_copy(
                    dstT[ti][:, t0 + tb * 512:t0 + (tb + 1) * 512], ps[:])

    def emit_v(th):
        for tt in range(8):
            gt = th * 8 + tt
            ps = psum_med.tile([128, 512], F32, tag="med")
            for dc in range(8):
                nc.tensor.matmul(
                    ps[:],
                    xT[:, dc, gt * 128:(gt + 1) * 128],
                    wv_t[:, dc, :],
                    start=(dc == 0), stop=(dc == 7),
                )
            nc.vector.tensor_copy(
                v3[gt][:, :, 0:DH],
                ps[:].rearrange("p (h d) -> p h d", h=HPC),
            )

    def head_work(qs, h):
        """Return (strip_closures, pv_closures) for one head's attention."""
        q0 = qs * QS_W
        n_kt = (q0 + QS_W) // 128
        ti, po = h // 2, (h % 2) * 64
        pts = []

        def mk_strip(kt):
            def go():
                c0 = max(0, kt * 128 - q0)
                sps = psum_big.tile([128, 1024], F32, tag="big")
                for s0, s1 in ((c0, 512), (max(c0, 512), 1024)):
                    if s0 >= s1:
                        continue
                    nc.tensor.matmul(
                        sps[:, s0:s1],
                        kT[ti][po:po + 64, kt * 128:(kt + 1) * 128],
                        qT[ti][po:po + 64, q0 + s0:q0 + s1],
                        start=True, stop=True,
                    )
                pt = ptpool.tile([128, QS_W], BF16, tag="pt",
                                 name=f"pt_{qs}_{h}_{kt}")
                nc.scalar.activation(
                    pt[:, c0:QS_W], sps[:, c0:QS_W],
                    mybir.ActivationFunctionType.Exp, scale=SCALE,
                )
                if kt * 128 >= q0:  # diagonal tile: mask within-chunk
                    nc.vector.tensor_mul(pt[:, c0:c0 + 128],
                                         pt[:, c0:c0 + 128], ltri[:])
                pts.append(pt)
            return go

        def mk_pv(qc):
            def go():
                qg = qs * 8 + qc
                cps = psum_cps.tile([128, DH + 1], F32, tag="cps")
                for kt in range(qg + 1):
                    nc.tensor.matmul(
                        cps[:],
                        pts[kt][:, qc * 128:(qc + 1) * 128],
                        v3[kt][:, h, :],
                        start=(kt == 0), stop=(kt == qg),
                    )
                rec = recpool.tile([128, 1], F32, tag="rec")
                nc.vector.reciprocal(rec[:], cps[:, DH:DH + 1])
                nc.vector.tensor_scalar_mul(
                    ctxc[qs][:, qc, h * DH:(h + 1) * DH], cps[:, 0:DH],
                    rec[:, 0:1])
            return go

        return [mk_strip(kt) for kt in range(n_kt)], \
               [mk_pv(qc) for qc in range(QS_W // 128)]

    def emit_out(qs, tts):
        q0 = qs * QS_W
        for qc in tts.get("xp", []):
            xp = psum_med.tile([128, 4, 128], BF16, tag="med")
            for ic in range(4):
                nc.tensor.transpose(xp[:, ic, :],
                                    ctxc[qs][:, qc, ic * 128:(ic + 1) * 128],
                                    ident[:])
            nc.vector.tensor_copy(
                ctxT[:, :, q0 + qc * 128:q0 + (qc + 1) * 128], xp[:])
        for tt in tts.get("o", []):
            osb = osbpool.tile([128, D], F32, tag="osb")
            for db in range(2):
                ops = psum_med.tile([128, 512], F32, tag="med")
                for ic in range(4):
                    nc.tensor.matmul(
                        ops[:],
                        ctxT[:, ic, tt * 128:(tt + 1) * 128],
                        wo_t[:, ic, db * 512:(db + 1) * 512],
                        start=(ic == 0), stop=(ic == 3),
                    )
                nc.vector.tensor_copy(osb[:, db * 512:(db + 1) * 512], ops[:])
            nc.sync.dma_start(o_view[tt], osb[:])

    # ---- emission ----
    # Program order must keep producers before consumers (the tile framework
    # tracks deps in emission order), so projection GEMM "filler" is emitted
    # at head boundaries -- but wrapped in high_priority with a large
    # NEGATIVE offset, which pushes its scheduler priority far BELOW all
    # attention work. The scheduler then runs filler only when attention is
    # blocked on ACT, keeping the PE saturated without delaying exps.
    # Within a head, S-strip/exp closures of head N interleave with the
    # PV/normalize closures of head N-1 so PV psum round trips never
    # head-of-line-block the S strips that feed ACT.
    LOW = -1_000_000

    def low(*fns):
        with tc.high_priority(offset=LOW):
            for fn in fns:
                fn()

    emit_qk(0, 0)  # head 0 needs this before anything else
    heads = [(0, h) for h in range(HPC)] + [(1, h) for h in range(HPC)]
    filler = {
        (0, 1): lambda: low(lambda: emit_v(0)),
        (0, 2): lambda: low(lambda: emit_qk(1, 0)),
        (0, 4): lambda: low(lambda: emit_qk(2, 0)),
        (0, 6): lambda: low(lambda: emit_qk(3, 0)),
        (1, 0): lambda: low(lambda: emit_qk(0, 1)),
        (1, 1): lambda: low(lambda: emit_v(1)),
        (1, 2): lambda: low(lambda: emit_qk(1, 1)),
        (1, 3): lambda: low(lambda: emit_out(0, {"xp": range(0, 8)})),
        (1, 4): lambda: low(lambda: emit_qk(2, 1)),
        (1, 5): lambda: low(lambda: emit_out(0, {"o": range(0, 4)})),
        (1, 6): lambda: low(lambda: emit_qk(3, 1)),
        (1, 7): lambda: low(lambda: emit_out(0, {"o": range(4, 8)})),
    }
    pending_pv = []
    for key in heads:
        if key in filler:
            filler[key]()
        strips, pvs = head_work(*key)
        k = 0
        for pv in pending_pv:
            if k < min(8, len(strips)):
                strips[k]()
                k += 1
            pv()
        for s in strips[k:]:
            s()
        pending_pv = pvs
    # last head's PV pipelined with the strip-1 output tail (one qc behind,
    # so the PE has output work ready while each normalize drains)
    for qc, pv in enumerate(pending_pv):
        pv()
        if qc > 0:
            emit_out(1, {"xp": [qc - 1], "o": [8 + qc - 1]})
    emit_out(1, {"xp": [7], "o": [15]})


def _build():
    from contextlib import ExitStack

    nc = bacc.Bacc("TRN2", target_bir_lowering=False, debug=False,
                   enable_asserts=True, num_devices=N_CORES)
    with tile.TileContext(nc) as tc:
        with ExitStack() as ctx:
            _emit(nc, tc, ctx)
    nc.compile()
    return nc


def _get_nc():
    if not _NC_CACHE:
        _NC_CACHE.append(_build())
    return _NC_CACHE[0]


def _in_maps(x, W_q, W_k, W_v, W_o):
    maps = []
    for c in range(N_CORES):
        b, g = c // 2, c % 2
        s = slice(g * I, (g + 1) * I)
        maps.append({
            "xt": np.ascontiguousarray(x[b].T).astype(NP_BF16),
            "wq": np.ascontiguousarray(W_q[:, s]).astype(NP_BF16),
            "wk": np.ascontiguousarray(W_k[:, s]).astype(NP_BF16),
            "wv": np.ascontiguousarray(W_v[:, s]).astype(NP_BF16),
            "wo": np.ascontiguousarray(W_o[s, :]).astype(NP_BF16),
        })
    return maps


def kernel(**inputs):
    x = np.asarray(inputs["x"], dtype=np.float32)
    W_q = np.asarray(inputs["W_q"], dtype=np.float32)
    W_k = np.asarray(inputs["W_k"], dtype=np.float32)
    W_v = np.asarray(inputs["W_v"], dtype=np.float32)
    W_o = np.asarray(inputs["W_o"], dtype=np.float32)

    nc = _get_nc()
    res = run_bass_kernel_spmd(nc, _in_maps(x, W_q, W_k, W_v, W_o),
                               core_ids=list(range(N_CORES)))
    out = np.empty((B, T, D), dtype=np.float32)
    for b in range(B):
        out[b] = res.results[2 * b]["o"] + res.results[2 * b + 1]["o"]
    return out

